# revision 1
# baseline (speedup 1.0000x reference)
"""Trainium2 Bass kernel for nn_Contracter (e3nn tensor product + message passing).

  reference:  x2_scatter = segment_sum(x2, idxs, N); x2g = x2_scatter[idxs]
              out[e,u,k] = sum_ij x1[e,u,i] * x2g[e,u,j] * ww3j[u,i,j,k]

  Sharding: edges sorted by destination node; each core owns a contiguous
  node range and all of its edges, so the segment-sum is fully core-local
  (no collectives).  Host pads each 128-node block's edges to whole
  128-edge chunks with an identical static structure on all 8 cores.

  Per core, one fused pass over node blocks:
    seg-sum:  one-hot (is_equal vs iota) matmuls accumulate
              table[n,(u,j)] over the block's edge chunks (PE); the same
              one-hots are PE-transposed and kept for the gather.
    Ctable:   PE-transpose table -> tableT, then block-diagonal weight
              matmuls give Ctable[n,(u,k,i)] = sum_j table[n,(u,j)] *
              ww3j[u,i,j,k] (tiny: scales with nodes, not edges).
              Column layout [A=(u,k,i0-3) | B=(u,k,i4-7) | C=(u,k,i8)]
              keeps every later elementwise op contiguous.
    per chunk: CG = onehotT @ Ctable[block] (PE gather, 6 bf16 matmuls),
              ACT copies PSUM->bf16, DVE forms TA/TB = x1 (broadcast
              over k) * CG_A/B at 2x, GpSimd forms TC and R4 = TA + TB,
              DVE reduces the 4 i-slots and adds TC, DMA out (bf16,
              host upcasts).
  One-hots are exact in bf16; inputs/products bf16, PSUM accumulation f32.
  Measured: ~670 us HW exec across 8 cores, rel absmax err ~7e-3.
"""
import sys
sys.path.insert(0, "/opt/trn_rl_repo")
import numpy as np
import ml_dtypes
import concourse.bass as bass
import concourse.bacc as bacc
import concourse.mybir as mybir
import concourse.tile as tile
from concourse import bass_utils
from concourse.masks import make_identity

P = 128
E = 100_000
N = 10_000
NCORES = 8
MUL, BD = 32, 9
DIM = MUL * BD            # 288
CDIM = MUL * BD * BD      # 2592
f32 = mybir.dt.float32
bf16 = mybir.dt.bfloat16
BF = ml_dtypes.bfloat16

UGROUPS = [(0, 11), (11, 22), (22, 32)]
UP = 22                   # product split: u < UP on DVE
UA = 10                   # add1 split: u < UA on DVE

_CACHE = {}


# ----------------------------------------------------------------- host prep
def _plan(idxs, n_nodes=N):
    order = np.argsort(idxs, kind="stable")
    deg = np.bincount(idxs, minlength=n_nodes)
    cum = np.concatenate([[0], np.cumsum(deg)])
    n_bounds = [0]
    for c in range(1, NCORES):
        n_bounds.append(int(np.searchsorted(cum, c * len(idxs) / NCORES)))
    n_bounds.append(n_nodes)
    cores = [dict(n_lo=n_bounds[c], n_hi=n_bounds[c + 1]) for c in range(NCORES)]
    NB = int(np.ceil(max(cr["n_hi"] - cr["n_lo"] for cr in cores) / P))
    CPB = np.zeros(NB, dtype=int)
    for cr in cores:
        n_lo, n_hi = cr["n_lo"], cr["n_hi"]
        for b in range(NB):
            blo, bhi = n_lo + b * P, min(n_lo + (b + 1) * P, n_hi)
            cnt = int(cum[bhi] - cum[blo]) if blo < n_hi else 0
            CPB[b] = max(CPB[b], (cnt + P - 1) // P)
    CPB = np.maximum(CPB, 1)
    return dict(order=order, cum=cum, cores=cores, NB=NB, CPB=CPB,
                E_pad=int(P * CPB.sum()))


def _core_arrays(plan, idxs, x1, x2):
    NB, CPB, E_pad = plan["NB"], plan["CPB"], plan["E_pad"]
    order, cum = plan["order"], plan["cum"]
    n_chunks = E_pad // P
    per_core = []
    for cr in plan["cores"]:
        n_lo, n_hi = cr["n_lo"], cr["n_hi"]
        x1s = np.zeros((E_pad, MUL * 10), BF)
        x2s = np.zeros((E_pad, DIM), BF)
        idxf = np.zeros(E_pad, np.float32)
        src = np.full(E_pad, -1, np.int64)
        pos = 0
        for b in range(NB):
            blo, bhi = n_lo + b * P, min(n_lo + (b + 1) * P, n_hi)
            se, ee = (int(cum[blo]), int(cum[bhi])) if blo < n_hi else (0, 0)
            sl = order[se:ee]
            cnt = ee - se
            x1s[pos:pos + cnt] = np.pad(
                x1[sl].reshape(cnt, MUL, BD), ((0, 0), (0, 0), (0, 1))
            ).reshape(cnt, MUL * 10).astype(BF)
            x2s[pos:pos + cnt] = x2[sl].astype(BF)
            idxf[pos:pos + cnt] = (idxs[sl] - blo).astype(np.float32)
            src[pos:pos + cnt] = sl
            pos += P * int(CPB[b])
        # idxfT[p, c] = idxf[c*128 + p]
        idxfT = np.ascontiguousarray(idxf.reshape(n_chunks, P).T)
        per_core.append(dict(x1s=x1s, x2s=x2s, idxfT=idxfT, src=src))
    return per_core


# Ctable global column layout: A=[(u,k,i0-3)] (1152) | B=[(u,k,i4-7)] (1152)
# | C=[(u,k,i8)] (288).  Per u-group the C-build matmuls emit the group's
# A/B/C slices; WW_g cols = [A_g (gu*36) | B_g (gu*36) | C_g (gu*9)].
A0, B0, C0 = 0, 1152, 2304


def _build_WW(w3j, weights):
    ww3j = np.einsum("up,pijk->uijk", weights, w3j).astype(np.float32)
    WW = np.zeros((DIM, 891), np.float32)
    for (u0, u1) in UGROUPS:
        gu = u1 - u0
        for u in range(u0, u1):
            blk = ww3j[u].transpose(1, 2, 0)          # [j, k, i]
            ul = u - u0
            WW[u * 9:(u + 1) * 9, ul * 36:(ul + 1) * 36] = \
                blk[:, :, 0:4].reshape(9, 36)
            WW[u * 9:(u + 1) * 9, gu * 36 + ul * 36:gu * 36 + (ul + 1) * 36] = \
                blk[:, :, 4:8].reshape(9, 36)
            WW[u * 9:(u + 1) * 9, gu * 72 + ul * 9:gu * 72 + (ul + 1) * 9] = \
                blk[:, :, 8].reshape(9, 9)
    return WW.astype(BF)


# ----------------------------------------------------------------- device
def _build_nc(NB, CPB, E_pad):
    NBN = NB * P
    n_chunks = E_pad // P
    nc = bacc.Bacc("TRN2", target_bir_lowering=False, debug=False,
                   num_devices=NCORES)
    d_x1 = nc.dram_tensor("x1s", [E_pad, MUL * 10], bf16, kind="ExternalInput")
    d_x2 = nc.dram_tensor("x2s", [E_pad, DIM], bf16, kind="ExternalInput")
    d_idxfT = nc.dram_tensor("idxfT", [P, n_chunks], f32, kind="ExternalInput")
    d_iota = nc.dram_tensor("iota", [P, P], bf16, kind="ExternalInput")
    d_WW = nc.dram_tensor("WW", [DIM, 891], bf16, kind="ExternalInput")
    d_out = nc.dram_tensor("out", [E_pad, DIM], bf16, kind="ExternalOutput")

    chunk_of = []
    for b in range(NB):
        chunk_of += [b] * int(CPB[b])

    with tile.TileContext(nc) as tc:
        with tc.tile_pool(name="persist", bufs=1) as pp:
            iota_t = pp.tile([P, P], bf16)
            nc.sync.dma_start(iota_t[:], d_iota[:])
            identb = pp.tile([P, P], bf16)
            make_identity(nc, identb[:])
            idxT = pp.tile([P, n_chunks], f32)
            nc.sync.dma_start(idxT[:], d_idxfT[:])
            WWt = []
            for gi, (u0, u1) in enumerate(UGROUPS):
                w = pp.tile([(u1 - u0) * 9, 891], bf16, tag=f"ww{gi}")
                nc.sync.dma_start(w[:], d_WW[u0 * 9:u1 * 9, :])
                WWt.append(w)
            tableT = {}
            for gi, (u0, u1) in enumerate(UGROUPS):
                for b in range(NB):
                    t = pp.tile([(u1 - u0) * 9, P], bf16, tag=f"tT{gi}_{b}")
                    tableT[(gi, b)] = t
            Ctab = []
            for b in range(NB):
                ct = pp.tile([P, CDIM], bf16, tag=f"ct{b}")
                Ctab.append(ct)

            ohTs = []
            for c in range(n_chunks):
                o = pp.tile([P, P], bf16, tag=f"ohT{c}")
                ohTs.append(o)

            # one fused pass: per block: seg-sum chunks (+ onehot transpose),
            # table transpose, Ctable build, then that block's sweep-2 chunks.
            with tc.tile_pool(name="wk", bufs=4) as wk, \
                 tc.tile_pool(name="wkb", bufs=3) as wkb, \
                 tc.tile_pool(name="wks", bufs=6) as wks, \
                 tc.tile_pool(name="pseg", bufs=1, space="PSUM") as pseg, \
                 tc.tile_pool(name="ptp", bufs=1, space="PSUM") as ptp, \
                 tc.tile_pool(name="pcg", bufs=1, space="PSUM") as pcg:
                ci = 0
                for b in range(NB):
                    nch = int(CPB[b])
                    # ---- sweep 1 for block b
                    seg = pseg.tile([P, 512], f32, tag="sg")
                    for k in range(nch):
                        c = ci + k
                        x2t = wk.tile([P, DIM], bf16, tag="x2")
                        nc.sync.dma_start(x2t[:], d_x2[c * P:(c + 1) * P, :])
                        oh = wk.tile([P, P], bf16, tag="oh")
                        nc.vector.tensor_scalar(
                            out=oh[:], in0=iota_t[:], scalar1=idxT[:, c:c + 1],
                            scalar2=None, op0=mybir.AluOpType.is_equal)
                        nc.tensor.matmul(seg[:, :DIM], lhsT=oh[:], rhs=x2t[:],
                                         start=(k == 0), stop=(k == nch - 1))
                        tpo = ptp.tile([P, 512], bf16, tag="tp")
                        nc.tensor.transpose(tpo[:, :P], oh[:], identb[:])
                        nc.scalar.copy(ohTs[c][:], tpo[:, :P])
                    ci += nch
                    tabs = wk.tile([P, DIM], bf16, tag="tab")
                    nc.scalar.copy(tabs[:], seg[:, :DIM])
                    for gi, (u0, u1) in enumerate(UGROUPS):
                        r = (u1 - u0) * 9
                        tp = ptp.tile([P, 512], bf16, tag="tp")
                        nc.tensor.transpose(tp[:r, :P], tabs[:, u0 * 9:u1 * 9],
                                            identb[:])
                        nc.scalar.copy(tableT[(gi, b)][:], tp[:r, :P])
                    # ---- Ctable build for block b (psum shared w/ transposes)
                    for gi, (u0, u1) in enumerate(UGROUPS):
                        gu = u1 - u0
                        spans = [(0, gu * 36, A0 + u0 * 36),
                                 (gu * 36, gu * 72, B0 + u0 * 36),
                                 (gu * 72, gu * 81, C0 + u0 * 9)]
                        for (n0, n1, dcol) in spans:
                            acc = ptp.tile([P, 512], f32, tag="tp")
                            nc.tensor.matmul(acc[:, :n1 - n0],
                                             lhsT=tableT[(gi, b)][:],
                                             rhs=WWt[gi][:, n0:n1],
                                             start=True, stop=True)
                            nc.scalar.copy(Ctab[b][:, dcol:dcol + n1 - n0],
                                           acc[:, :n1 - n0])
                    # ---- sweep 2 for block b
                    for c in range(ci - nch, ci):
                        x1b = wks.tile([P, MUL * 10], bf16, tag="x1b")
                        nc.sync.dma_start(x1b[:], d_x1[c * P:(c + 1) * P, :])
                        cgb = wkb.tile([P, CDIM], bf16, tag="cgb")
                        cg = pcg.tile([P, CDIM], f32, tag="cg")
                        for n0 in range(0, CDIM, 512):
                            n1 = min(n0 + 512, CDIM)
                            nc.tensor.matmul(cg[:, n0:n1], lhsT=ohTs[c][:],
                                             rhs=Ctab[b][:, n0:n1],
                                             start=True, stop=True)
                        nc.scalar.copy(cgb[:], cg[:])
                        TA = wkb.tile([P, 1152], bf16, tag="TA")
                        TB = wkb.tile([P, 1152], bf16, tag="TB")
                        TC = wks.tile([P, DIM], bf16, tag="TC")
                        x1b4 = x1b[:].rearrange("p (u k i) -> p u k i",
                                                u=MUL, k=1, i=10)
                        for eng, u_s, u_e in ((nc.vector, 0, MUL),):
                            nu = u_e - u_s
                            eng.tensor_tensor(
                                out=TA[:, u_s * 36:u_e * 36].rearrange(
                                    "p (u k i) -> p u k i", u=nu, k=BD),
                                in0=x1b4[:, u_s:u_e, :, 0:4].to_broadcast(
                                    [P, nu, BD, 4]),
                                in1=cgb[:, A0 + u_s * 36:A0 + u_e * 36].rearrange(
                                    "p (u k i) -> p u k i", u=nu, k=BD),
                                op=mybir.AluOpType.mult)
                            eng.tensor_tensor(
                                out=TB[:, u_s * 36:u_e * 36].rearrange(
                                    "p (u k i) -> p u k i", u=nu, k=BD),
                                in0=x1b4[:, u_s:u_e, :, 4:8].to_broadcast(
                                    [P, nu, BD, 4]),
                                in1=cgb[:, B0 + u_s * 36:B0 + u_e * 36].rearrange(
                                    "p (u k i) -> p u k i", u=nu, k=BD),
                                op=mybir.AluOpType.mult)
                        nc.gpsimd.tensor_tensor(
                            out=TC[:].rearrange("p (u k) -> p u k", u=MUL),
                            in0=x1b4[:, :, :, 8].to_broadcast([P, MUL, BD]),
                            in1=cgb[:, C0:C0 + DIM].rearrange(
                                "p (u k) -> p u k", u=MUL),
                            op=mybir.AluOpType.mult)
                        R4 = wkb.tile([P, 1152], bf16, tag="R4")
                        nc.gpsimd.tensor_tensor(out=R4[:], in0=TA[:],
                                                in1=TB[:],
                                                op=mybir.AluOpType.add)
                        R1 = wks.tile([P, DIM], bf16, tag="R1")
                        with nc.allow_low_precision(reason="bf16 i-reduce"):
                            nc.vector.tensor_reduce(
                                out=R1[:].rearrange("p (u k) -> p u k", u=MUL),
                                in_=R4[:].rearrange("p (u k i) -> p u k i",
                                                    u=MUL, k=BD),
                                axis=mybir.AxisListType.X,
                                op=mybir.AluOpType.add)
                        outt = wks.tile([P, DIM], bf16, tag="outt")
                        nc.vector.tensor_tensor(out=outt[:], in0=R1[:],
                                                in1=TC[:],
                                                op=mybir.AluOpType.add)
                        nc.sync.dma_start(d_out[c * P:(c + 1) * P, :], outt[:])
    nc.compile()
    return nc


# ----------------------------------------------------------------- entry
def kernel(x1, x2, idxs, scatter_dim_size, w3j, weights):
    x1 = np.asarray(x1, dtype=np.float32)
    x2 = np.asarray(x2, dtype=np.float32)
    idxs_np = np.asarray(idxs).astype(np.int64)
    w3j = np.asarray(w3j, dtype=np.float32)
    weights = np.asarray(weights, dtype=np.float32)

    plan = _plan(idxs_np, int(scatter_dim_size))
    per_core = _core_arrays(plan, idxs_np, x1, x2)
    WW = _build_WW(w3j, weights)
    iota = np.broadcast_to(np.arange(P, dtype=np.float32)[None, :],
                           (P, P)).astype(BF)

    key = (plan["NB"], tuple(plan["CPB"]), plan["E_pad"])
    if key not in _CACHE:
        _CACHE[key] = _build_nc(plan["NB"], plan["CPB"], plan["E_pad"])
    nc = _CACHE[key]

    in_maps = [{"x1s": pc["x1s"], "x2s": pc["x2s"], "idxfT": pc["idxfT"],
                "iota": iota, "WW": WW} for pc in per_core]
    res = None
    for attempt in range(3):
        try:
            res = bass_utils.run_bass_kernel_spmd(nc, in_maps,
                                                  core_ids=list(range(NCORES)))
            break
        except Exception:
            if attempt == 2:
                raise
            import time as _time
            _time.sleep(5)
    out = np.zeros((E, DIM), np.float32)
    for pc, r in zip(per_core, res.results):
        real = pc["src"] >= 0
        out[pc["src"][real]] = r["out"][real].astype(np.float32)
    return out.reshape(E, MUL, BD)


if __name__ == "__main__":
    sys.path.insert(0, "/root/problem")
    import reference as ref
    import jax
    with jax.default_device(jax.devices("cpu")[0]):
        inputs = {k: np.asarray(v) if hasattr(v, "shape") else v
                  for k, v in ref.setup_inputs().items()}
    got = kernel(**inputs)
    print("kernel done", got.shape)



# revision 14
# speedup vs baseline: 1.0803x; 1.0803x over previous
"""Trainium2 Bass kernel for nn_Contracter (e3nn tensor product + message passing).

  reference:  x2_scatter = segment_sum(x2, idxs, N); x2g = x2_scatter[idxs]
              out[e,u,k] = sum_ij x1[e,u,i] * x2g[e,u,j] * ww3j[u,i,j,k]

  Sharding: edges sorted by destination node; each core owns a contiguous
  node range and all of its edges, so the segment-sum is fully core-local
  (no collectives).  Host pads each 128-node block's edges to whole
  128-edge chunks with an identical static structure on all 8 cores.

  v2 pipeline per core (one fused pass over node blocks):
    sweep1:   oh[e,n]=is_equal(iota, idx) on GpSimd; tableT_g[(u,j),n]
              accumulated FEATURE-major on PE (lhsT=x2 chunk, rhs=oh) --
              no transposes; ohT produced by DMA-transpose (XBAR).
    tail:     tableT drained (DVE), Ctab[n,(u:72|k:8|i)] built by PE from
              WW (block-diag ww3j), drained ACT/DVE to bf16 SBUF.
    sweep2:   CG = ohT^T @ Ctab gather in 4 PSUM pieces (PE, bf16);
              pieces drained by ACT to bf16 while DVE does the broadcast
              mult x1*CG at 2x (u<28), GpSimd covers u>=28; the i=8 piece
              is multiplied by DVE straight out of PSUM (drain fused);
              i-reduction as a log tree: T4,T2 on DVE (2x), T1 + (+TC)
              on GpSimd.  DMAs batched 4 chunks at a time.
"""
import sys
sys.path.insert(0, "/opt/trn_rl_repo")
import numpy as np
import ml_dtypes
import concourse.bass as bass
import concourse.bacc as bacc
import concourse.mybir as mybir
import concourse.tile as tile
from concourse import bass_utils

P = 128
E = 100_000
N = 10_000
NCORES = 8
MUL, BD = 32, 9
DIM = MUL * BD            # 288
CDIM = MUL * BD * BD      # 2592
AB = MUL * BD * 8         # 2304  (u,k,i0..7) cols: u*72 + k*8 + i
C0 = AB                   # C cols: C0 + u*9 + k
f32 = mybir.dt.float32
bf16 = mybir.dt.bfloat16
BF = ml_dtypes.bfloat16

UGROUPS = [(0, 11), (11, 22), (22, 32)]
USPL = 28                 # DVE mult covers u<USPL, GpSimd the rest
P1 = 14 * 72              # 1008: CG piece 1 (u0..13)
P2 = 28 * 72              # 2016: CG piece 2 (u14..27)

_CACHE = {}


# ----------------------------------------------------------------- host prep
def _plan(idxs, n_nodes=N):
    order = np.argsort(idxs, kind="stable")
    deg = np.bincount(idxs, minlength=n_nodes)
    cum = np.concatenate([[0], np.cumsum(deg)])
    n_bounds = [0]
    for c in range(1, NCORES):
        n_bounds.append(int(np.searchsorted(cum, c * len(idxs) / NCORES)))
    n_bounds.append(n_nodes)
    cores = [dict(n_lo=n_bounds[c], n_hi=n_bounds[c + 1]) for c in range(NCORES)]
    NB = int(np.ceil(max(cr["n_hi"] - cr["n_lo"] for cr in cores) / P))
    CPB = np.zeros(NB, dtype=int)
    for cr in cores:
        n_lo, n_hi = cr["n_lo"], cr["n_hi"]
        for b in range(NB):
            blo, bhi = n_lo + b * P, min(n_lo + (b + 1) * P, n_hi)
            cnt = int(cum[bhi] - cum[blo]) if blo < n_hi else 0
            CPB[b] = max(CPB[b], (cnt + P - 1) // P)
    CPB = np.maximum(CPB, 1)
    return dict(order=order, cum=cum, cores=cores, NB=NB, CPB=CPB,
                E_pad=int(P * CPB.sum()))


def _core_arrays(plan, idxs, x1, x2):
    NB, CPB, E_pad = plan["NB"], plan["CPB"], plan["E_pad"]
    order, cum = plan["order"], plan["cum"]
    n_chunks = E_pad // P
    per_core = []
    for cr in plan["cores"]:
        n_lo, n_hi = cr["n_lo"], cr["n_hi"]
        x1s = np.zeros((E_pad, MUL * 10), BF)
        x2s = np.zeros((E_pad, DIM), BF)
        idxf = np.zeros(E_pad, np.float32)
        src = np.full(E_pad, -1, np.int64)
        pos = 0
        for b in range(NB):
            blo, bhi = n_lo + b * P, min(n_lo + (b + 1) * P, n_hi)
            se, ee = (int(cum[blo]), int(cum[bhi])) if blo < n_hi else (0, 0)
            sl = order[se:ee]
            cnt = ee - se
            x1s[pos:pos + cnt] = np.pad(
                x1[sl].reshape(cnt, MUL, BD), ((0, 0), (0, 0), (0, 1))
            ).reshape(cnt, MUL * 10).astype(BF)
            x2s[pos:pos + cnt] = x2[sl].astype(BF)
            idxf[pos:pos + cnt] = (idxs[sl] - blo).astype(np.float32)
            src[pos:pos + cnt] = sl
            pos += P * int(CPB[b])
        idxfT = np.ascontiguousarray(idxf.reshape(n_chunks, P).T)
        per_core.append(dict(x1s=x1s, x2s=x2s, idxfT=idxfT, src=src))
    return per_core


def _build_WW(w3j, weights):
    # WW[(u*9+j), group-local cols]: AB cols ul*72 + k*8 + i (i<8),
    # C cols gu*72 + ul*9 + k.
    ww3j = np.einsum("up,pijk->uijk", weights, w3j).astype(np.float32)
    WW = np.zeros((DIM, 891), np.float32)
    for (u0, u1) in UGROUPS:
        gu = u1 - u0
        for u in range(u0, u1):
            ul = u - u0
            blk = ww3j[u]                                  # [i, j, k]
            for j in range(BD):
                row = u * 9 + j
                WW[row, ul * 72:(ul + 1) * 72] = \
                    blk[0:8, j, :].T.reshape(72)           # k*8 + i
                WW[row, gu * 72 + ul * 9:gu * 72 + (ul + 1) * 9] = blk[8, j, :]
    return WW.astype(BF)


# ----------------------------------------------------------------- device
def _build_nc(NB, CPB, E_pad):
    n_chunks = E_pad // P
    nc = bacc.Bacc("TRN2", target_bir_lowering=False, debug=False,
                   num_devices=NCORES)
    d_x1 = nc.dram_tensor("x1s", [E_pad, MUL * 10], bf16, kind="ExternalInput")
    d_x2 = nc.dram_tensor("x2s", [E_pad, DIM], bf16, kind="ExternalInput")
    d_idxfT = nc.dram_tensor("idxfT", [P, n_chunks], f32, kind="ExternalInput")
    d_iota = nc.dram_tensor("iota", [P, P], bf16, kind="ExternalInput")
    d_WW = nc.dram_tensor("WW", [DIM, 891], bf16, kind="ExternalInput")
    d_out = nc.dram_tensor("out", [E_pad, DIM], bf16, kind="ExternalOutput")

    MM = mybir.AluOpType.mult
    AD = mybir.AluOpType.add
    EQ = mybir.AluOpType.is_equal

    with tile.TileContext(nc) as tc:
        with tc.tile_pool(name="persist", bufs=1) as pp:
            iota_t = pp.tile([P, P], bf16)
            nc.sync.dma_start(iota_t[:], d_iota[:])
            idxT = pp.tile([P, n_chunks], f32)
            nc.sync.dma_start(idxT[:], d_idxfT[:])
            WWt = []
            for gi, (u0, u1) in enumerate(UGROUPS):
                gu = u1 - u0
                w = pp.tile([gu * 9, gu * 81], bf16, tag=f"ww{gi}")
                nc.sync.dma_start(w[:], d_WW[u0 * 9:u1 * 9, :gu * 81])
                WWt.append(w)
            Ctab = []
            for b in range(NB):
                ctb = pp.tile([P, CDIM], bf16, tag=f"ct{b}")
                Ctab.append(ctb)
            ohTs = []
            for c in range(n_chunks):
                ohc = pp.tile([P, P], bf16, tag=f"ohT{c}")
                ohTs.append(ohc)

            # PSUM banks (8 x 2KB): ptab 1, pct 2, pcg 2, pcg3 2, pcgc 1
            with tc.tile_pool(name="wx", bufs=2) as wx, \
                 tc.tile_pool(name="woh", bufs=3) as woh, \
                 tc.tile_pool(name="wtt", bufs=2) as wtt, \
                 tc.tile_pool(name="wcg", bufs=2) as wcg, \
                 tc.tile_pool(name="wtp", bufs=2) as wtp, \
                 tc.tile_pool(name="ptab", bufs=1, space="PSUM") as ptab, \
                 tc.tile_pool(name="pct", bufs=1, space="PSUM") as pct, \
                 tc.tile_pool(name="pcg", bufs=1, space="PSUM") as pcg, \
                 tc.tile_pool(name="pcg3", bufs=2, space="PSUM") as pcg3, \
                 tc.tile_pool(name="pcgc", bufs=1, space="PSUM") as pcgc:
                ci = 0
                for b in range(NB):
                    nch = int(CPB[b])
                    # ---------------- sweep 1: segment sum (feature-major)
                    ptgt = ptab.tile([99, 3 * P], f32, tag="tt")
                    for k in range(nch):
                        c = ci + k
                        if c % 4 == 0:
                            g4 = min(4, n_chunks - c)
                            x2t = wx.tile([P, 4 * DIM], bf16, tag="x2")
                            nc.sync.dma_start(
                                x2t[:, :g4 * DIM].rearrange(
                                    "p (j f) -> p j f", f=DIM),
                                d_x2[c * P:(c + g4) * P, :].rearrange(
                                    "(j p) f -> p j f", p=P))
                        oh = woh.tile([P, P], bf16, tag="oh")
                        nc.gpsimd.tensor_scalar(
                            out=oh[:], in0=iota_t[:], scalar1=idxT[:, c:c + 1],
                            scalar2=None, op0=EQ)
                        j4 = (c % 4) * DIM
                        for g, (u0, u1) in enumerate(UGROUPS):
                            # one whole-bank has_written clear per block:
                            # start=True clears the bits for the WHOLE bank,
                            # so only the block's first matmul may set it.
                            nc.tensor.matmul(
                                ptgt[0:(u1 - u0) * 9, g * P:(g + 1) * P],
                                lhsT=x2t[:, j4 + u0 * 9:j4 + u1 * 9],
                                rhs=oh[:], start=(k == 0 and g == 0),
                                stop=(k == nch - 1),
                                skip_group_check=True)
                        nc.sync.dma_start_transpose(ohTs[c][:], oh[:])
                    ci += nch
                    # ---------------- tail: drain tableT, build Ctab
                    tts = wtt.tile([99, 3 * P], bf16, tag="tb")
                    nc.vector.tensor_scalar(
                        out=tts[:], in0=ptgt[:], scalar1=1.0, scalar2=None,
                        op0=MM)
                    for g, (u0, u1) in enumerate(UGROUPS):
                        gu = u1 - u0
                        ct = pct.tile([P, 891], f32, tag="ct")
                        for n0 in range(0, gu * 81, 512):
                            n1 = min(n0 + 512, gu * 81)
                            nc.tensor.matmul(
                                ct[:, n0:n1],
                                lhsT=tts[0:gu * 9, g * P:(g + 1) * P],
                                rhs=WWt[g][:, n0:n1], start=True, stop=True)
                        nc.scalar.copy(Ctab[b][:, u0 * 72:u1 * 72],
                                       ct[:, :gu * 72])
                        nc.vector.tensor_scalar(
                            out=Ctab[b][:, C0 + u0 * 9:C0 + u1 * 9],
                            in0=ct[:, gu * 72:gu * 81], scalar1=1.0,
                            scalar2=None, op0=MM)
                    # ---------------- sweep 2: gather + contract
                    for c in range(ci - nch, ci):
                        if c % 4 == 0:
                            g4 = min(4, n_chunks - c)
                            x1t = wx.tile([P, 4 * MUL * 10], bf16, tag="x1")
                            nc.sync.dma_start(
                                x1t[:, :g4 * MUL * 10].rearrange(
                                    "p (j f) -> p j f", f=MUL * 10),
                                d_x1[c * P:(c + g4) * P, :].rearrange(
                                    "(j p) f -> p j f", p=P))
                            obuf = wx.tile([P, 4 * DIM], bf16, tag="ob")
                        j1 = (c % 4) * MUL * 10
                        x14 = x1t[:, j1:j1 + MUL * 10].rearrange(
                            "p (u k i) -> p u k i", u=MUL, k=1, i=10)
                        # gather pieces
                        cg1 = pcg.tile([P, P1], f32, tag="cg")
                        nc.tensor.matmul(cg1[:, :512], lhsT=ohTs[c][:],
                                         rhs=Ctab[b][:, :512],
                                         start=True, stop=True)
                        nc.tensor.matmul(cg1[:, 512:P1], lhsT=ohTs[c][:],
                                         rhs=Ctab[b][:, 512:P1],
                                         start=True, stop=True)
                        cg2 = pcg.tile([P, P1], f32, tag="cg")
                        nc.tensor.matmul(cg2[:, :512], lhsT=ohTs[c][:],
                                         rhs=Ctab[b][:, P1:P1 + 512],
                                         start=True, stop=True)
                        nc.tensor.matmul(cg2[:, 512:P1], lhsT=ohTs[c][:],
                                         rhs=Ctab[b][:, P1 + 512:P2],
                                         start=True, stop=True)
                        cg3 = pcg3.tile([P, AB - P2], f32, tag="cg3")
                        nc.tensor.matmul(cg3[:], lhsT=ohTs[c][:],
                                         rhs=Ctab[b][:, P2:AB],
                                         start=True, stop=True)
                        cgc = pcgc.tile([P, DIM], f32, tag="cgc")
                        nc.tensor.matmul(cgc[:], lhsT=ohTs[c][:],
                                         rhs=Ctab[b][:, AB:CDIM],
                                         start=True, stop=True)
                        # drains (ACT) + fused drain-mult for C (DVE)
                        cgb12 = wcg.tile([P, P2], bf16, tag="cgb12")
                        nc.scalar.copy(cgb12[:, :P1], cg1[:])
                        nc.scalar.copy(cgb12[:, P1:P2], cg2[:])
                        cgb3 = wcg.tile([P, AB - P2], bf16, tag="cgb3")
                        nc.scalar.copy(cgb3[:], cg3[:])
                        TCt = wtp.tile([P, DIM], bf16, tag="TC")
                        nc.vector.tensor_tensor(
                            out=TCt[:].rearrange("p (u k i) -> p u k i",
                                                 u=MUL, k=BD),
                            in0=x14[:, :, :, 8:9].to_broadcast([P, MUL, BD, 1]),
                            in1=cgc[:].rearrange("p (u k i) -> p u k i",
                                                 u=MUL, k=BD),
                            op=MM)
                        # broadcast mult: DVE u<28 (2x), GpSimd u28..31
                        TP = wtp.tile([P, AB], bf16, tag="TP")
                        nc.vector.tensor_tensor(
                            out=TP[:, :P2].rearrange("p (u k i) -> p u k i",
                                                     u=USPL, k=BD),
                            in0=x14[:, :USPL, :, 0:8].to_broadcast(
                                [P, USPL, BD, 8]),
                            in1=cgb12[:].rearrange("p (u k i) -> p u k i",
                                                   u=USPL, k=BD),
                            op=MM)
                        nc.gpsimd.tensor_tensor(
                            out=TP[:, P2:AB].rearrange("p (u k i) -> p u k i",
                                                       u=MUL - USPL, k=BD),
                            in0=x14[:, USPL:, :, 0:8].to_broadcast(
                                [P, MUL - USPL, BD, 8]),
                            in1=cgb3[:].rearrange("p (u k i) -> p u k i",
                                                  u=MUL - USPL, k=BD),
                            op=MM)
                        # i-reduction tree
                        T4 = wtp.tile([P, DIM * 4], bf16, tag="T4")
                        tp8 = TP[:].rearrange("p (uk i) -> p uk i", i=8)
                        nc.vector.tensor_tensor(
                            out=T4[:].rearrange("p (uk i) -> p uk i", i=4),
                            in0=tp8[:, :, 0:4], in1=tp8[:, :, 4:8], op=AD)
                        T2 = wtp.tile([P, DIM * 2], bf16, tag="T2")
                        t44 = T4[:].rearrange("p (uk i) -> p uk i", i=4)
                        nc.vector.tensor_tensor(
                            out=T2[:].rearrange("p (uk i) -> p uk i", i=2),
                            in0=t44[:, :, 0:2], in1=t44[:, :, 2:4], op=AD)
                        T1 = wtp.tile([P, DIM], bf16, tag="T1")
                        t22 = T2[:].rearrange("p (uk i) -> p uk i", i=2)
                        nc.gpsimd.tensor_tensor(
                            out=T1[:].rearrange("p (uk i) -> p uk i", i=1),
                            in0=t22[:, :, 0:1], in1=t22[:, :, 1:2], op=AD)
                        jo = (c % 4) * DIM
                        nc.gpsimd.tensor_tensor(
                            out=obuf[:, jo:jo + DIM], in0=T1[:], in1=TCt[:],
                            op=AD)
                        if c % 4 == 3 or c == n_chunks - 1:
                            c0 = (c // 4) * 4
                            g4 = c - c0 + 1
                            nc.sync.dma_start(
                                d_out[c0 * P:(c0 + g4) * P, :].rearrange(
                                    "(j p) f -> p j f", p=P),
                                obuf[:, :g4 * DIM].rearrange(
                                    "p (j f) -> p j f", f=DIM))
    nc.compile()
    return nc


# ----------------------------------------------------------------- entry
def kernel(x1, x2, idxs, scatter_dim_size, w3j, weights):
    x1 = np.asarray(x1, dtype=np.float32)
    x2 = np.asarray(x2, dtype=np.float32)
    idxs_np = np.asarray(idxs).astype(np.int64)
    w3j = np.asarray(w3j, dtype=np.float32)
    weights = np.asarray(weights, dtype=np.float32)

    plan = _plan(idxs_np, int(scatter_dim_size))
    per_core = _core_arrays(plan, idxs_np, x1, x2)
    WW = _build_WW(w3j, weights)
    iota = np.broadcast_to(np.arange(P, dtype=np.float32)[None, :],
                           (P, P)).astype(BF)

    key = (plan["NB"], tuple(plan["CPB"]), plan["E_pad"])
    if key not in _CACHE:
        _CACHE[key] = _build_nc(plan["NB"], plan["CPB"], plan["E_pad"])
    nc = _CACHE[key]

    in_maps = [{"x1s": pc["x1s"], "x2s": pc["x2s"], "idxfT": pc["idxfT"],
                "iota": iota, "WW": WW} for pc in per_core]
    res = None
    for attempt in range(3):
        try:
            res = bass_utils.run_bass_kernel_spmd(nc, in_maps,
                                                  core_ids=list(range(NCORES)))
            break
        except Exception:
            if attempt == 2:
                raise
            import time as _time
            _time.sleep(5)
    out = np.zeros((E, DIM), np.float32)
    for pc, r in zip(per_core, res.results):
        real = pc["src"] >= 0
        out[pc["src"][real]] = r["out"][real].astype(np.float32)
    return out.reshape(E, MUL, BD)


if __name__ == "__main__":
    sys.path.insert(0, "/root/problem")
    import reference as ref
    import jax
    with jax.default_device(jax.devices("cpu")[0]):
        inputs = {k: np.asarray(v) if hasattr(v, "shape") else v
                  for k, v in ref.setup_inputs().items()}
    got = kernel(**inputs)
    print("kernel done", got.shape)


# revision 17
# speedup vs baseline: 1.4714x; 1.3621x over previous
"""Trainium2 Bass kernel for nn_Contracter (e3nn tensor product + message passing).

  reference:  x2_scatter = segment_sum(x2, idxs, N); x2g = x2_scatter[idxs]
              out[e,u,k] = sum_ij x1[e,u,i] * x2g[e,u,j] * ww3j[u,i,j,k]

  Sharding: edges sorted by destination node; each core owns a contiguous
  node range and all of its edges, so the segment-sum is fully core-local
  (no collectives).  Host pads each 128-node block's edges to whole
  128-edge chunks with an identical static structure on all 8 cores.

  v3 pipeline per core (one fused pass over node blocks):
    group load (4 chunks): x1/x2/idxB batched DMAs; one DVE tensor_tensor
              builds 4 edge-major one-hots (iota vs idx), one DVE
              tensor_scalar builds 4 node-major one-hots from the
              host-replicated idxB -- no transposes, no per-chunk ops.
    sweep1:   seg-sum accumulated on PE (lhsT=oh chunk, rhs=x2 chunk).
    tail:     table transposed on PE per u-group, Ctab[n, (u:72|k:8|i)]
              built by PE from WW (block-diag ww3j), drained ACT/DVE.
    sweep2:   CG = ohT^T @ Ctab gather in 4 PSUM pieces (PE, bf16);
              AB pieces drained by ACT to bf16 while DVE does the
              broadcast mult x1*CG at 2x (u<28), GpSimd covers u>=28;
              the i=8 piece is multiplied by DVE straight out of PSUM;
              i-reduction log tree: T4,T2 on DVE (2x), T1 and +TC on
              GpSimd.  Outputs staged and written 4 chunks per DMA.
"""
import sys
sys.path.insert(0, "/opt/trn_rl_repo")
import numpy as np
import ml_dtypes
import concourse.bass as bass
import concourse.bacc as bacc
import concourse.mybir as mybir
import concourse.tile as tile
from concourse import bass_utils
from concourse.masks import make_identity

P = 128
E = 100_000
N = 10_000
NCORES = 8
MUL, BD = 32, 9
DIM = MUL * BD            # 288
CDIM = MUL * BD * BD      # 2592
AB = MUL * BD * 8         # 2304  (u,k,i0..7) cols: u*72 + k*8 + i
C0 = AB                   # C cols: C0 + u*9 + k
f32 = mybir.dt.float32
bf16 = mybir.dt.bfloat16
BF = ml_dtypes.bfloat16

UGROUPS = [(0, 11), (11, 22), (22, 32)]
USPL = 28                 # DVE mult covers u<USPL, GpSimd the rest
P1 = 14 * 72              # 1008: CG piece 1 (u0..13)
P2 = 28 * 72              # 2016: CG piece 2 (u14..27)

_CACHE = {}


# ----------------------------------------------------------------- host prep
def _plan(idxs, n_nodes=N):
    order = np.argsort(idxs, kind="stable")
    deg = np.bincount(idxs, minlength=n_nodes)
    cum = np.concatenate([[0], np.cumsum(deg)])
    n_bounds = [0]
    for c in range(1, NCORES):
        n_bounds.append(int(np.searchsorted(cum, c * len(idxs) / NCORES)))
    n_bounds.append(n_nodes)
    cores = [dict(n_lo=n_bounds[c], n_hi=n_bounds[c + 1]) for c in range(NCORES)]
    NB = int(np.ceil(max(cr["n_hi"] - cr["n_lo"] for cr in cores) / P))
    CPB = np.zeros(NB, dtype=int)
    for cr in cores:
        n_lo, n_hi = cr["n_lo"], cr["n_hi"]
        for b in range(NB):
            blo, bhi = n_lo + b * P, min(n_lo + (b + 1) * P, n_hi)
            cnt = int(cum[bhi] - cum[blo]) if blo < n_hi else 0
            CPB[b] = max(CPB[b], (cnt + P - 1) // P)
    CPB = np.maximum(CPB, 1)
    return dict(order=order, cum=cum, cores=cores, NB=NB, CPB=CPB,
                E_pad=int(P * CPB.sum()))


def _core_arrays(plan, idxs, x1, x2):
    NB, CPB, E_pad = plan["NB"], plan["CPB"], plan["E_pad"]
    order, cum = plan["order"], plan["cum"]
    n_chunks = E_pad // P
    per_core = []
    for cr in plan["cores"]:
        n_lo, n_hi = cr["n_lo"], cr["n_hi"]
        x1s = np.zeros((E_pad, MUL * 10), BF)
        x2s = np.zeros((E_pad, DIM), BF)
        idxf = np.zeros(E_pad, np.float32)
        src = np.full(E_pad, -1, np.int64)
        pos = 0
        for b in range(NB):
            blo, bhi = n_lo + b * P, min(n_lo + (b + 1) * P, n_hi)
            se, ee = (int(cum[blo]), int(cum[bhi])) if blo < n_hi else (0, 0)
            sl = order[se:ee]
            cnt = ee - se
            x1s[pos:pos + cnt] = np.pad(
                x1[sl].reshape(cnt, MUL, BD), ((0, 0), (0, 0), (0, 1))
            ).reshape(cnt, MUL * 10).astype(BF)
            x2s[pos:pos + cnt] = x2[sl].astype(BF)
            idxf[pos:pos + cnt] = (idxs[sl] - blo).astype(np.float32)
            src[pos:pos + cnt] = sl
            pos += P * int(CPB[b])
        idxfT = np.ascontiguousarray(idxf.reshape(n_chunks, P).T)
        # idxB[p, c*128+e] = idxf[c*128+e]  (replicated down partitions)
        idxB = np.ascontiguousarray(
            np.broadcast_to(idxf[None, :], (P, E_pad)).astype(BF))
        per_core.append(dict(x1s=x1s, x2s=x2s, idxfT=idxfT, idxB=idxB,
                             src=src))
    return per_core


def _build_WW(w3j, weights):
    # WW[(u*9+j), group-local cols]: AB cols ul*72 + k*8 + i (i<8),
    # C cols gu*72 + ul*9 + k.
    ww3j = np.einsum("up,pijk->uijk", weights, w3j).astype(np.float32)
    WW = np.zeros((DIM, 891), np.float32)
    for (u0, u1) in UGROUPS:
        gu = u1 - u0
        for u in range(u0, u1):
            ul = u - u0
            blk = ww3j[u]                                  # [i, j, k]
            for j in range(BD):
                row = u * 9 + j
                WW[row, ul * 72:(ul + 1) * 72] = \
                    blk[0:8, j, :].T.reshape(72)           # k*8 + i
                WW[row, gu * 72 + ul * 9:gu * 72 + (ul + 1) * 9] = blk[8, j, :]
    return WW.astype(BF)


# ----------------------------------------------------------------- device
def _build_nc(NB, CPB, E_pad):
    n_chunks = E_pad // P
    n_grp = (n_chunks + 3) // 4
    nc = bacc.Bacc("TRN2", target_bir_lowering=False, debug=False,
                   num_devices=NCORES)
    d_x1 = nc.dram_tensor("x1s", [E_pad, MUL * 10], bf16, kind="ExternalInput")
    d_x2 = nc.dram_tensor("x2s", [E_pad, DIM], bf16, kind="ExternalInput")
    d_idxfT = nc.dram_tensor("idxfT", [P, n_chunks], f32, kind="ExternalInput")
    d_idxB = nc.dram_tensor("idxB", [P, E_pad], bf16, kind="ExternalInput")
    d_iota4 = nc.dram_tensor("iota4", [P, 4 * P], bf16, kind="ExternalInput")
    d_iotaC = nc.dram_tensor("iotaC", [P, 1], f32, kind="ExternalInput")
    d_WW = nc.dram_tensor("WW", [DIM, 891], bf16, kind="ExternalInput")
    d_out = nc.dram_tensor("out", [E_pad, DIM], bf16, kind="ExternalOutput")

    MM = mybir.AluOpType.mult
    AD = mybir.AluOpType.add
    EQ = mybir.AluOpType.is_equal

    with tile.TileContext(nc) as tc:
        with tc.tile_pool(name="persist", bufs=1) as pp:
            iota4 = pp.tile([P, 4 * P], bf16)
            nc.sync.dma_start(iota4[:], d_iota4[:])
            iotaC = pp.tile([P, 1], f32)
            nc.sync.dma_start(iotaC[:], d_iotaC[:])
            idxT = pp.tile([P, n_chunks], f32)
            nc.sync.dma_start(idxT[:], d_idxfT[:])
            identb = pp.tile([P, P], bf16)
            make_identity(nc, identb[:])
            WWt = []
            for gi, (u0, u1) in enumerate(UGROUPS):
                gu = u1 - u0
                w = pp.tile([gu * 9, gu * 81], bf16, tag=f"ww{gi}")
                nc.sync.dma_start(w[:], d_WW[u0 * 9:u1 * 9, :gu * 81])
                WWt.append(w)
            Ctab = []
            for b in range(NB):
                ctb = pp.tile([P, CDIM], bf16, tag=f"ct{b}")
                Ctab.append(ctb)
            ohTs = []
            for g in range(n_grp):
                ohg = pp.tile([P, 4 * P], bf16, tag=f"ohT{g}")
                ohTs.append(ohg)

            # PSUM banks (8 x 2KB):
            #   pseg 1, ptp 1, pct 1, pcg 2, pcg3 2, pcgc 1  -> 8
            with tc.tile_pool(name="wx", bufs=2) as wx, \
                 tc.tile_pool(name="woh", bufs=4) as woh, \
                 tc.tile_pool(name="wtt", bufs=2) as wtt, \
                 tc.tile_pool(name="wcg", bufs=2) as wcg, \
                 tc.tile_pool(name="wtp", bufs=2) as wtp, \
                 tc.tile_pool(name="pseg", bufs=1, space="PSUM") as pseg, \
                 tc.tile_pool(name="ptp", bufs=1, space="PSUM") as ptp, \
                 tc.tile_pool(name="pct", bufs=1, space="PSUM") as pct, \
                 tc.tile_pool(name="pcg", bufs=1, space="PSUM") as pcg, \
                 tc.tile_pool(name="pcg3", bufs=2, space="PSUM") as pcg3, \
                 tc.tile_pool(name="pcgc", bufs=1, space="PSUM") as pcgc:
                ci = 0
                for b in range(NB):
                    nch = int(CPB[b])
                    # ---------------- sweep 1: one-hots + segment-sum
                    seg = pseg.tile([P, DIM], f32, tag="sg")
                    for k in range(nch):
                        c = ci + k
                        if c % 4 == 0:
                            g4 = min(4, n_chunks - c)
                            x2t = wx.tile([P, 4 * DIM], bf16, tag="x2")
                            nc.sync.dma_start(
                                x2t[:, :g4 * DIM].rearrange(
                                    "p (j f) -> p j f", f=DIM),
                                d_x2[c * P:(c + g4) * P, :].rearrange(
                                    "(j p) f -> p j f", p=P))
                            idxB4 = wx.tile([P, 4 * P], bf16, tag="ib")
                            nc.sync.dma_start(idxB4[:, :g4 * P],
                                              d_idxB[:, c * P:(c + g4) * P])
                            oh4 = woh.tile([P, 4 * P], bf16, tag="oh")
                            nc.vector.tensor_tensor(
                                out=oh4[:, :g4 * P].rearrange(
                                    "p (j n) -> p j n", n=P),
                                in0=iota4[:, :g4 * P].rearrange(
                                    "p (j n) -> p j n", n=P),
                                in1=idxT[:, c:c + g4].rearrange(
                                    "p (j o) -> p j o", o=1).to_broadcast(
                                        [P, g4, P]),
                                op=EQ)
                            nc.vector.tensor_scalar(
                                out=ohTs[c // 4][:, :g4 * P],
                                in0=idxB4[:, :g4 * P], scalar1=iotaC[:, 0:1],
                                scalar2=None, op0=EQ)
                        j4 = (c % 4) * DIM
                        nc.tensor.matmul(
                            seg[:], lhsT=oh4[:, (c % 4) * P:(c % 4 + 1) * P],
                            rhs=x2t[:, j4:j4 + DIM],
                            start=(k == 0), stop=(k == nch - 1))
                    ci += nch
                    # ---------------- tail: transpose table, build Ctab
                    tabs = wtt.tile([P, DIM], bf16, tag="tab")
                    nc.scalar.copy(tabs[:], seg[:])
                    tts = []
                    for g, (u0, u1) in enumerate(UGROUPS):
                        gu = u1 - u0
                        tp = ptp.tile([99, P], bf16, tag="tp")
                        nc.tensor.transpose(tp[:gu * 9, :],
                                            tabs[:, u0 * 9:u1 * 9], identb[:])
                        tt = wtt.tile([99, P], bf16, tag=f"tb{g}")
                        nc.vector.tensor_scalar(
                            out=tt[:gu * 9, :], in0=tp[:gu * 9, :],
                            scalar1=1.0, scalar2=None, op0=MM)
                        tts.append(tt)
                    for g, (u0, u1) in enumerate(UGROUPS):
                        gu = u1 - u0
                        for n0 in range(0, gu * 81, 512):
                            n1 = min(n0 + 512, gu * 81)
                            ct = pct.tile([P, 512], f32, tag="ct")
                            nc.tensor.matmul(ct[:, :n1 - n0],
                                             lhsT=tts[g][:gu * 9, :],
                                             rhs=WWt[g][:, n0:n1],
                                             start=True, stop=True)
                            if n1 <= gu * 72:
                                nc.scalar.copy(
                                    Ctab[b][:, u0 * 72 + n0:u0 * 72 + n1],
                                    ct[:, :n1 - n0])
                            elif n0 >= gu * 72:
                                nc.vector.tensor_scalar(
                                    out=Ctab[b][:, C0 + u0 * 9 + (n0 - gu * 72):
                                                C0 + u0 * 9 + (n1 - gu * 72)],
                                    in0=ct[:, :n1 - n0], scalar1=1.0,
                                    scalar2=None, op0=MM)
                            else:
                                h = gu * 72 - n0
                                nc.scalar.copy(
                                    Ctab[b][:, u0 * 72 + n0:u1 * 72],
                                    ct[:, :h])
                                nc.vector.tensor_scalar(
                                    out=Ctab[b][:, C0 + u0 * 9:
                                                C0 + u0 * 9 + (n1 - n0 - h)],
                                    in0=ct[:, h:n1 - n0], scalar1=1.0,
                                    scalar2=None, op0=MM)
                    # ---------------- sweep 2: gather + contract
                    for c in range(ci - nch, ci):
                        if c % 4 == 0:
                            g4 = min(4, n_chunks - c)
                            x1t = wx.tile([P, 4 * MUL * 10], bf16, tag="x1")
                            nc.sync.dma_start(
                                x1t[:, :g4 * MUL * 10].rearrange(
                                    "p (j f) -> p j f", f=MUL * 10),
                                d_x1[c * P:(c + g4) * P, :].rearrange(
                                    "(j p) f -> p j f", p=P))
                            obuf = wx.tile([P, 4 * DIM], bf16, tag="ob")
                        j1 = (c % 4) * MUL * 10
                        x14 = x1t[:, j1:j1 + MUL * 10].rearrange(
                            "p (u k i) -> p u k i", u=MUL, k=1, i=10)
                        ohT = ohTs[c // 4][:, (c % 4) * P:(c % 4 + 1) * P]
                        # gather pieces (order chosen to fill WAR stalls)
                        cg1 = pcg.tile([P, P1], f32, tag="cg")
                        nc.tensor.matmul(cg1[:, :512], lhsT=ohT,
                                         rhs=Ctab[b][:, :512],
                                         start=True, stop=True)
                        nc.tensor.matmul(cg1[:, 512:P1], lhsT=ohT,
                                         rhs=Ctab[b][:, 512:P1],
                                         start=True, stop=True)
                        cgb12 = wcg.tile([P, P2], bf16, tag="cgb12")
                        nc.scalar.copy(cgb12[:, :P1], cg1[:])
                        cg3 = pcg3.tile([P, AB - P2], f32, tag="cg3")
                        nc.tensor.matmul(cg3[:], lhsT=ohT,
                                         rhs=Ctab[b][:, P2:AB],
                                         start=True, stop=True)
                        cgb3 = wcg.tile([P, AB - P2], bf16, tag="cgb3")
                        nc.scalar.copy(cgb3[:], cg3[:])
                        cgc = pcgc.tile([P, DIM], f32, tag="cgc")
                        nc.tensor.matmul(cgc[:], lhsT=ohT,
                                         rhs=Ctab[b][:, AB:CDIM],
                                         start=True, stop=True)
                        TCt = wtp.tile([P, DIM], bf16, tag="TC")
                        nc.vector.tensor_tensor(
                            out=TCt[:].rearrange("p (u k i) -> p u k i",
                                                 u=MUL, k=BD),
                            in0=x14[:, :, :, 8:9].to_broadcast([P, MUL, BD, 1]),
                            in1=cgc[:].rearrange("p (u k i) -> p u k i",
                                                 u=MUL, k=BD),
                            op=MM)
                        cg2 = pcg.tile([P, P1], f32, tag="cg")
                        nc.tensor.matmul(cg2[:, :512], lhsT=ohT,
                                         rhs=Ctab[b][:, P1:P1 + 512],
                                         start=True, stop=True)
                        nc.tensor.matmul(cg2[:, 512:P1], lhsT=ohT,
                                         rhs=Ctab[b][:, P1 + 512:P2],
                                         start=True, stop=True)
                        nc.scalar.copy(cgb12[:, P1:P2], cg2[:])
                        # broadcast mult: DVE u<28 (2x), GpSimd u28..31
                        TP = wtp.tile([P, AB], bf16, tag="TP")
                        nc.vector.tensor_tensor(
                            out=TP[:, :P2].rearrange("p (u k i) -> p u k i",
                                                     u=USPL, k=BD),
                            in0=x14[:, :USPL, :, 0:8].to_broadcast(
                                [P, USPL, BD, 8]),
                            in1=cgb12[:].rearrange("p (u k i) -> p u k i",
                                                   u=USPL, k=BD),
                            op=MM)
                        nc.gpsimd.tensor_tensor(
                            out=TP[:, P2:AB].rearrange("p (u k i) -> p u k i",
                                                       u=MUL - USPL, k=BD),
                            in0=x14[:, USPL:, :, 0:8].to_broadcast(
                                [P, MUL - USPL, BD, 8]),
                            in1=cgb3[:].rearrange("p (u k i) -> p u k i",
                                                  u=MUL - USPL, k=BD),
                            op=MM)
                        # i-reduction tree
                        T4 = wtp.tile([P, DIM * 4], bf16, tag="T4")
                        tp8 = TP[:].rearrange("p (uk i) -> p uk i", i=8)
                        nc.vector.tensor_tensor(
                            out=T4[:].rearrange("p (uk i) -> p uk i", i=4),
                            in0=tp8[:, :, 0:4], in1=tp8[:, :, 4:8], op=AD)
                        T2 = wtp.tile([P, DIM * 2], bf16, tag="T2")
                        t44 = T4[:].rearrange("p (uk i) -> p uk i", i=4)
                        nc.vector.tensor_tensor(
                            out=T2[:].rearrange("p (uk i) -> p uk i", i=2),
                            in0=t44[:, :, 0:2], in1=t44[:, :, 2:4], op=AD)
                        T1 = wtp.tile([P, DIM], bf16, tag="T1")
                        t22 = T2[:].rearrange("p (uk i) -> p uk i", i=2)
                        nc.gpsimd.tensor_tensor(
                            out=T1[:].rearrange("p (uk i) -> p uk i", i=1),
                            in0=t22[:, :, 0:1], in1=t22[:, :, 1:2], op=AD)
                        jo = (c % 4) * DIM
                        nc.gpsimd.tensor_tensor(
                            out=obuf[:, jo:jo + DIM], in0=T1[:], in1=TCt[:],
                            op=AD)
                        if c % 4 == 3 or c == n_chunks - 1:
                            c0 = (c // 4) * 4
                            g4 = c - c0 + 1
                            nc.sync.dma_start(
                                d_out[c0 * P:(c0 + g4) * P, :].rearrange(
                                    "(j p) f -> p j f", p=P),
                                obuf[:, :g4 * DIM].rearrange(
                                    "p (j f) -> p j f", f=DIM))
    nc.compile()
    return nc


# ----------------------------------------------------------------- entry
def kernel(x1, x2, idxs, scatter_dim_size, w3j, weights):
    x1 = np.asarray(x1, dtype=np.float32)
    x2 = np.asarray(x2, dtype=np.float32)
    idxs_np = np.asarray(idxs).astype(np.int64)
    w3j = np.asarray(w3j, dtype=np.float32)
    weights = np.asarray(weights, dtype=np.float32)

    plan = _plan(idxs_np, int(scatter_dim_size))
    per_core = _core_arrays(plan, idxs_np, x1, x2)
    WW = _build_WW(w3j, weights)
    iota1 = np.arange(P, dtype=np.float32)
    iota4 = np.ascontiguousarray(
        np.broadcast_to(np.tile(iota1, 4)[None, :], (P, 4 * P)).astype(BF))
    iotaC = np.ascontiguousarray(iota1[:, None].astype(np.float32))

    key = (plan["NB"], tuple(plan["CPB"]), plan["E_pad"])
    if key not in _CACHE:
        _CACHE[key] = _build_nc(plan["NB"], plan["CPB"], plan["E_pad"])
    nc = _CACHE[key]

    in_maps = [{"x1s": pc["x1s"], "x2s": pc["x2s"], "idxfT": pc["idxfT"],
                "idxB": pc["idxB"], "iota4": iota4, "iotaC": iotaC,
                "WW": WW} for pc in per_core]
    res = None
    for attempt in range(3):
        try:
            res = bass_utils.run_bass_kernel_spmd(nc, in_maps,
                                                  core_ids=list(range(NCORES)))
            break
        except Exception:
            if attempt == 2:
                raise
            import time as _time
            _time.sleep(5)
    out = np.zeros((E, DIM), np.float32)
    for pc, r in zip(per_core, res.results):
        real = pc["src"] >= 0
        out[pc["src"][real]] = r["out"][real].astype(np.float32)
    return out.reshape(E, MUL, BD)


if __name__ == "__main__":
    sys.path.insert(0, "/root/problem")
    import reference as ref
    import jax
    with jax.default_device(jax.devices("cpu")[0]):
        inputs = {k: np.asarray(v) if hasattr(v, "shape") else v
                  for k, v in ref.setup_inputs().items()}
    got = kernel(**inputs)
    print("kernel done", got.shape)
